# revision 1
# baseline (speedup 1.0000x reference)
"""Trainium2 Bass kernel for nn_LongRangeFeaturizer (Ewald sum featurizer).

Shards the 16 independent systems across 8 NeuronCores (2 systems/core).
All heavy math (charges matmul, k-space structure factors, trig, short-range
erf/cutoff coefficients, scatter, final combine) runs on-device.
"""

import sys

sys.path.insert(0, "/opt/trn_rl_repo")

import numpy as np

import concourse.bass as bass
import concourse.mybir as mybir
import concourse.tile as tile
from concourse import bacc, bass_utils

dt = mybir.dt
F32, F16, I16 = dt.float32, dt.float16, dt.int16
AF = mybir.ActivationFunctionType
AOP = mybir.AluOpType

PI = float(np.pi)
MAGIC = float(1.5 * 2**23)  # round-to-nearest-int magic constant for fp32

# Problem constants
S, N, D, E = 16, 512, 64, 16384
LCELL = 8.0
SMEAR = 1.0
EXCL = 5.0
LRWL = 1.0
PREF = 1.0
NMAX = 8
NCORES = 8
SYS_PER_CORE = S // NCORES

_CACHE = {}


def _half_kgrid():
    r = np.arange(-NMAX, NMAX + 1)
    n = np.stack(np.meshgrid(r, r, r, indexing="ij"), -1).reshape(-1, 3)
    n = n[np.any(n != 0, axis=1)]
    nsq = (n * n).sum(1)
    kcut2 = (2.0 * PI / LRWL) ** 2
    ks = (2.0 * PI / LCELL) ** 2 * nsq  # cubic cell L
    keep = ks <= kcut2
    n = n[keep]
    pos = (n[:, 0] > 0) | ((n[:, 0] == 0) & (n[:, 1] > 0)) | (
        (n[:, 0] == 0) & (n[:, 1] == 0) & (n[:, 2] > 0)
    )
    return n[pos].astype(np.int64)  # [K, 3]


def _sr_arrange(nidx, ndist):
    """Group edges by source j; slot targets i per row with duplicate-i layering.

    Returns list of (D_arr[S,N,R_l] f32, I_arr[S,N,R_l] i16) per layer."""
    layers_d = []  # per layer: dict-free dense arrays
    layers_i = []
    # first pass: compute per-edge (system, j, i, layer, slot)
    all_rows = []
    Lmax = 0
    for s in range(S):
        i_t = nidx[s, :, 0].astype(np.int64)
        j_t = nidx[s, :, 1].astype(np.int64)
        d_t = ndist[s].astype(np.float64)
        cidx = j_t * N + i_t
        order = np.argsort(cidx, kind="stable")
        cs, ds_ = cidx[order], d_t[order]
        # occurrence index within identical cidx runs
        first = np.concatenate([[0], np.nonzero(np.diff(cs))[0] + 1])
        run_id = np.zeros(E, np.int64)
        run_id[first] = 1
        run_id = np.cumsum(run_id) - 1
        occ = np.arange(E) - first[run_id]
        all_rows.append((cs // N, cs % N, ds_, occ))
        Lmax = max(Lmax, int(occ.max()) + 1)
    # R per layer
    Rs = []
    for lay in range(Lmax):
        r_need = 0
        for s in range(S):
            js, is_, ds_, occ = all_rows[s]
            m = occ == lay
            if m.sum() == 0:
                continue
            cnt = np.bincount(js[m], minlength=N)
            r_need = max(r_need, int(cnt.max()))
        r_need = max(2, r_need + (r_need % 2))  # even
        Rs.append(r_need)
    for lay in range(Lmax):
        R = Rs[lay]
        Da = np.full((S, N, R), 1.0e6, np.float32)  # pad distance -> sr masked to 0
        Ia = np.full((S, N, R), -1, np.int16)
        for s in range(S):
            js, is_, ds_, occ = all_rows[s]
            m = occ == lay
            jm, im, dm = js[m], is_[m], ds_[m]
            # slot position within each j row (edges sorted by cidx -> grouped by j)
            cnt = np.bincount(jm, minlength=N)
            start = np.concatenate([[0], np.cumsum(cnt)[:-1]])
            slot = np.arange(len(jm)) - start[jm]
            Da[s, jm, slot] = dm.astype(np.float32)
            Ia[s, jm, slot] = im.astype(np.int16)
        layers_d.append(Da)
        layers_i.append(Ia)
    return layers_d, layers_i, Rs


def _build_nc(K, Rs, reps=1):
    """Build the per-core SPMD program. K = number of half-grid k vectors."""
    nc = bacc.Bacc("TRN2", target_bir_lowering=False, debug=False,
                   num_devices=NCORES)

    # const APs for activation biases
    for val in (PI / 2,):
        t = nc.alloc_sbuf_tensor(f"constap-{val}", [128, 1], F32)
        nc.gpsimd.memset(t.ap(), val)
        nc.const_aps.aps[(F32, val)] = t.ap()
    nc.all_engine_barrier()

    def din(name, shape, d=F32):
        return nc.dram_tensor(name, shape, d, kind="ExternalInput").ap()

    SC = SYS_PER_CORE
    featT = din("featT", [D + 1, SC * N])          # [65, 1024] f32
    pT6 = din("pT6", [6, SC * N], F16)             # fp16 hi/lo frac positions
    WT = din("WT", [D + 1, D])                     # [65, 64] f32 (W.T ; b)
    nt6 = din("nt6", [6, K], F16)                  # [n;n] fp16
    KT0 = (K + 127) // 128
    Gcol = din("Gcol", [128, KT0])                 # f32, k-tile-major columns
    G16row = din("G16row", [D, K], F16)
    negI = din("negI", [128, 128], F16)
    id16 = din("id16", [128, 128], F16)
    id32 = din("id32", [128, 128])
    NBLK = SC * (N // 128)
    srd = [din(f"srd{l}", [128, NBLK * Rs[l]]) for l in range(len(Rs))]
    sri = [din(f"sri{l}", [128, NBLK * Rs[l]], I16) for l in range(len(Rs))]
    out = nc.dram_tensor("out", [SC * D, N], F32, kind="ExternalOutput").ap()

    NT = N // 128            # 4 atom tiles
    KT = (K + 127) // 128    # 9 k tiles
    kw = [min(128, K - 128 * t) for t in range(KT)]
    chunks = []
    c0 = 0
    while c0 < K:
        w = min(512, K - c0)
        chunks.append((c0, w))
        c0 += w

    selfc = PREF * float(np.sqrt(2.0 / PI) / SMEAR)
    bgov = PREF * float(PI * SMEAR**2 / (LCELL**3))

    from contextlib import nullcontext
    with tile.TileContext(nc) as tc:
        with (
            tc.tile_pool(name="const", bufs=1) as cp,
            tc.tile_pool(name="work", bufs=2) as wp,
            tc.tile_pool(name="trig", bufs=1) as tp,
            tc.tile_pool(name="psum", bufs=2, space="PSUM") as pp,
            tc.For_i(0, reps, 1) if reps > 1 else nullcontext(),
        ):
            # ---- constants ----
            t_WT = cp.tile([D + 1, D], F32, tag="wt")
            nc.sync.dma_start(out=t_WT[:], in_=WT[:])
            t_nt6 = cp.tile([6, K], F16, tag="nt6")
            nc.sync.dma_start(out=t_nt6[:], in_=nt6[:])
            t_G = cp.tile([128, KT0], F32, tag="g")
            nc.sync.dma_start(out=t_G[:], in_=Gcol[:])
            t_G16r = cp.tile([D, K], F16, tag="g16r")
            nc.sync.dma_start(out=t_G16r[:], in_=G16row[:])
            t_negI = cp.tile([128, 128], F16, tag="negi")
            nc.sync.dma_start(out=t_negI[:], in_=negI[:])
            t_id16 = cp.tile([128, 128], F16, tag="id16")
            nc.sync.dma_start(out=t_id16[:], in_=id16[:])
            t_id32 = cp.tile([128, 128], F32, tag="id32")
            nc.sync.dma_start(out=t_id32[:], in_=id32[:])
            t_feat = cp.tile([D + 1, SC * N], F32, tag="feat")
            nc.sync.dma_start(out=t_feat[:], in_=featT[:])
            t_pT6 = cp.tile([6, SC * N], F16, tag="p6")
            nc.sync.dma_start(out=t_pT6[:], in_=pT6[:])

            # ---- SR coefficients, batched over all systems/j-tiles ----
            erf_insts = []
            sin_insts = []
            sr16_all = []
            sr_tiles = []
            for l, R in enumerate(Rs):
                WL = NBLK * R
                t_d = cp.tile([128, WL], F32, tag=f"srd{l}")
                nc.sync.dma_start(out=t_d[:], in_=srd[l][:])
                t_erf = wp.tile([128, WL], F32, tag=f"srerf{l}")
                ei = nc.scalar.activation(t_erf[:], t_d[:], AF.Erf,
                                          scale=float(1 / np.sqrt(2.0)))
                erf_insts.append(ei.ins)
                sr_tiles.append((t_d, t_erf))
            for l, R in enumerate(Rs):
                WL = NBLK * R
                t_d, t_erf = sr_tiles[l]
                t_rec = wp.tile([128, WL], F32, tag=f"srrec{l}")
                nc.vector.reciprocal(t_rec[:], t_d[:])
                t_msk = wp.tile([128, WL], F32, tag=f"srmsk{l}")
                nc.vector.tensor_scalar(out=t_msk[:], in0=t_d[:],
                                        scalar1=EXCL, scalar2=-PREF,
                                        op0=AOP.is_lt, op1=AOP.mult)
                t_fc = wp.tile([128, WL], F32, tag=f"srfc{l}")
                si = nc.scalar.activation(t_fc[:], t_d[:], AF.Sin,
                                          scale=float(PI / EXCL), bias=PI / 2)
                sin_insts.append(si.ins)
                nc.vector.tensor_scalar(out=t_fc[:], in0=t_fc[:],
                                        scalar1=0.5, scalar2=0.5,
                                        op0=AOP.mult, op1=AOP.add)
                nc.vector.tensor_tensor(out=t_erf[:], in0=t_erf[:],
                                        in1=t_rec[:], op=AOP.mult)
                nc.vector.tensor_tensor(out=t_erf[:], in0=t_erf[:],
                                        in1=t_msk[:], op=AOP.mult)
                t_sr16 = cp.tile([128, WL], F16, tag=f"sr16{l}")
                nc.vector.tensor_tensor(out=t_sr16[:], in0=t_erf[:],
                                        in1=t_fc[:], op=AOP.mult)
                sr16_all.append(t_sr16)
            idx_all = []
            for l, R in enumerate(Rs):
                t_ia = cp.tile([128, NBLK * R], I16, tag=f"sriall{l}")
                nc.sync.dma_start(out=t_ia[:], in_=sri[l][:])
                idx_all.append(t_ia)
            mt_tiles = {}
            for sys in range(SC):
                for jt in range(NT):
                    blk = sys * NT + jt
                    mt_layers = []
                    for l, R in enumerate(Rs):
                        csl_b = slice(blk * R, blk * R + R)
                        t_m = wp.tile([128, N], F16, tag=f"mt{l}")
                        nc.gpsimd.local_scatter(out_ap=t_m[:],
                                                data_ap=sr16_all[l][:, csl_b],
                                                idxs_ap=idx_all[l][:, csl_b],
                                                channels=128,
                                                num_elems=N, num_idxs=R)
                        mt_layers.append(t_m)
                    t_acc = tp.tile([128, N], F16, tag=f"mtacc{sys}_{jt}")
                    if len(mt_layers) == 1:
                        nc.vector.tensor_copy(out=t_acc[:], in_=mt_layers[0][:])
                    else:
                        nc.vector.tensor_tensor(out=t_acc[:], in0=mt_layers[0][:],
                                                in1=mt_layers[1][:], op=AOP.add)
                        for l in range(2, len(mt_layers)):
                            nc.vector.tensor_tensor(out=t_acc[:], in0=t_acc[:],
                                                    in1=mt_layers[l][:], op=AOP.add)
                    mt_tiles[(sys, jt)] = t_acc

            # ---- KN-layout trig for BOTH systems at once: cT,sT [K, 2N] ----
            kn_c, kn_s = [], []
            for kt in range(KT):
                w = kw[kt]
                ksl = slice(kt * 128, kt * 128 + w)
                ps_uT = pp.tile([128, SC * N], F32, tag="big3")
                for h in range(SC):
                    hsl = slice(h * N, h * N + N)
                    nc.tensor.matmul(out=ps_uT[:w, hsl], lhsT=t_nt6[:, ksl],
                                     rhs=t_pT6[:, hsl], start=True, stop=False)
                t_i16k = wp.tile([128, SC * N], F16, tag="i16kn")
                nc.vector.tensor_scalar(out=t_i16k[:w], in0=ps_uT[:w],
                                        scalar1=MAGIC, scalar2=MAGIC,
                                        op0=AOP.add, op1=AOP.subtract)
                for h in range(SC):
                    hsl = slice(h * N, h * N + N)
                    nc.tensor.matmul(out=ps_uT[:w, hsl], lhsT=t_negI[:w, :w],
                                     rhs=t_i16k[:w, hsl], start=False, stop=True)
                t_s2 = tp.tile([128, SC * N], F16, tag=f"skn{kt}")
                sin_insts.append(nc.scalar.activation(
                    t_s2[:w], ps_uT[:w], AF.Sin, scale=2 * PI).ins)
                t_ra2 = wp.tile([128, SC * N], F32, tag="rabskn")
                sin_insts.append(nc.scalar.activation(
                    t_ra2[:w], ps_uT[:w], AF.Abs).ins)
                t_c2 = tp.tile([128, SC * N], F16, tag=f"ckn{kt}")
                sin_insts.append(nc.scalar.activation(
                    t_c2[:w], t_ra2[:w], AF.Sin,
                    scale=-2 * PI, bias=PI / 2).ins)
                kn_s.append(t_s2)
                kn_c.append(t_c2)

            sysdat = {}
            for sys in range(SC):
                r0 = sys * N
                csl = slice(sys * N, sys * N + N)

                # ---- charges ----
                ps_qT = pp.tile([D, N], F32, tag="one")
                nc.tensor.matmul(out=ps_qT[:], lhsT=t_WT[:], rhs=t_feat[:, csl],
                                 start=True, stop=True)
                t_qT = tp.tile([D, N], F32, tag=f"qT{sys}")
                nc.vector.tensor_copy(out=t_qT[:], in_=ps_qT[:])
                t_q16 = []
                for nt_i in range(NT):
                    fsl = slice(sys * N + nt_i * 128, sys * N + nt_i * 128 + 128)
                    ps_q = pp.tile([128, D], F32, tag="one")
                    nc.tensor.matmul(out=ps_q[:], lhsT=t_feat[:, fsl], rhs=t_WT[:],
                                     start=True, stop=True)
                    tq = tp.tile([128, D], F16, tag=f"q16_{sys}_{nt_i}")
                    nc.vector.tensor_copy(out=tq[:], in_=ps_q[:])
                    t_q16.append(tq)

                t_MT = [mt_tiles[(sys, jt)] for jt in range(NT)]

                # ---- NK-layout trig: c,s [N, K] fp16 ----
                t_c_nk, t_s_nk = [], []
                for nt_i in range(NT):
                    psl = slice(sys * N + nt_i * 128, sys * N + nt_i * 128 + 128)
                    ps_u = pp.tile([128, K], F32, tag="big3")
                    for (c0, w) in chunks:
                        nc.tensor.matmul(out=ps_u[:, c0:c0 + w],
                                         lhsT=t_pT6[:, psl],
                                         rhs=t_nt6[:, c0:c0 + w],
                                         start=True, stop=False)
                    t_i16 = wp.tile([128, K], F16, tag="i16nk")
                    nc.vector.tensor_scalar(out=t_i16[:], in0=ps_u[:],
                                            scalar1=MAGIC, scalar2=MAGIC,
                                            op0=AOP.add, op1=AOP.subtract)
                    for (c0, w) in chunks:
                        nc.tensor.matmul(out=ps_u[:, c0:c0 + w], lhsT=t_negI[:],
                                         rhs=t_i16[:, c0:c0 + w],
                                         start=False, stop=True)
                    t_s = tp.tile([128, K], F16, tag=f"snk{sys}_{nt_i}")
                    sin_insts.append(nc.scalar.activation(
                        t_s[:], ps_u[:], AF.Sin, scale=2 * PI).ins)
                    t_ra = wp.tile([128, K], F32, tag="rabsnk")
                    sin_insts.append(nc.scalar.activation(
                        t_ra[:], ps_u[:], AF.Abs).ins)
                    t_c = tp.tile([128, K], F16, tag=f"cnk{sys}_{nt_i}")
                    sin_insts.append(nc.scalar.activation(
                        t_c[:], t_ra[:], AF.Sin, scale=-2 * PI,
                        bias=PI / 2).ins)
                    t_s_nk.append(t_s)
                    t_c_nk.append(t_c)

                sysdat[sys] = (t_qT, t_q16, t_MT, t_c_nk, t_s_nk)

            for sys in range(SC):
                r0 = sys * N
                csl = slice(sys * N, sys * N + N)
                t_qT, t_q16, t_MT, t_c_nk, t_s_nk = sysdat[sys]
                # ---- stage1: ScT/SsT [64, K] fp32 psum ----
                ps_S = pp.tile([128, K], F32, tag="big3")
                ps_ScT = ps_S[0:D]
                ps_SsT = ps_S[D:2 * D]
                for nt_i in range(NT):
                    st, sp = nt_i == 0, nt_i == NT - 1
                    for (c0, w) in chunks:
                        nc.tensor.matmul(out=ps_ScT[:, c0:c0 + w],
                                         lhsT=t_q16[nt_i][:],
                                         rhs=t_c_nk[nt_i][:, c0:c0 + w],
                                         start=st, stop=sp)
                        nc.tensor.matmul(out=ps_SsT[:, c0:c0 + w],
                                         lhsT=t_q16[nt_i][:],
                                         rhs=t_s_nk[nt_i][:, c0:c0 + w],
                                         start=st, stop=sp)
                t_ScT = wp.tile([D, K], F16, tag="sct")
                nc.vector.tensor_tensor(out=t_ScT[:], in0=ps_ScT[:],
                                        in1=t_G16r[:],
                                        op=AOP.mult)
                t_SsT = wp.tile([D, K], F16, tag="sst")
                nc.vector.tensor_tensor(out=t_SsT[:], in0=ps_SsT[:],
                                        in1=t_G16r[:],
                                        op=AOP.mult)

                # ---- transposes: GSc/GSs [K, 64] fp16, 4 k-tiles per bank ----
                t_GSc, t_GSs = [], []
                for (srct, dst_list, tg) in ((t_ScT, t_GSc, f"gsc{sys}"),
                                             (t_SsT, t_GSs, f"gss{sys}")):
                    for g0 in range(0, KT, 4):
                        gn = min(4, KT - g0)
                        ps_tr = pp.tile([128, gn * D], F16, tag="one")
                        for gi in range(gn):
                            kt = g0 + gi
                            w = kw[kt]
                            ksl = slice(kt * 128, kt * 128 + w)
                            nc.tensor.transpose(
                                out=ps_tr[:w, gi * D:gi * D + D],
                                in_=srct[:, ksl], identity=t_id16[:D, :D])
                        t_g = tp.tile([128, gn * D], F16, tag=f"{tg}{g0}")
                        nc.vector.tensor_copy(out=t_g[:], in_=ps_tr[:])
                        for gi in range(gn):
                            dst_list.append(t_g[:, gi * D:gi * D + D])

                # ---- stage2 + M@q into one PSUM ----
                ps_pot = pp.tile([D, N], F32, tag="big3")
                for kt in range(KT):
                    w = kw[kt]
                    nc.tensor.matmul(out=ps_pot[:], lhsT=t_GSc[kt][:w],
                                     rhs=kn_c[kt][:w, csl], start=(kt == 0),
                                     stop=False)
                    nc.tensor.matmul(out=ps_pot[:], lhsT=t_GSs[kt][:w],
                                     rhs=kn_s[kt][:w, csl], start=False,
                                     stop=False)
                for jt in range(NT):
                    nc.tensor.matmul(out=ps_pot[:], lhsT=t_q16[jt][:],
                                     rhs=t_MT[jt][:], start=False,
                                     stop=(jt == NT - 1))

                # ---- combine + output ----
                t_sum = wp.tile([D, 1], F32, tag="sumq")
                nc.vector.reduce_sum(t_sum[:], t_qT[:], axis=mybir.AxisListType.X)
                nc.vector.tensor_scalar(out=t_sum[:], in0=t_sum[:], scalar1=bgov,
                                        scalar2=None, op0=AOP.mult)
                t_sc = wp.tile([D, N], F32, tag="qsc")
                nc.vector.tensor_scalar(out=t_sc[:], in0=t_qT[:], scalar1=selfc,
                                        scalar2=None, op0=AOP.mult)
                t_pot = wp.tile([D, N], F32, tag="potf")
                nc.vector.tensor_tensor(out=t_pot[:], in0=ps_pot[:], in1=t_sc[:],
                                        op=AOP.subtract)
                nc.vector.tensor_scalar(out=t_pot[:], in0=t_pot[:],
                                        scalar1=t_sum[:, :1], scalar2=None,
                                        op0=AOP.subtract)
                nc.vector.tensor_tensor(out=t_pot[:], in0=t_pot[:], in1=t_qT[:],
                                        op=AOP.mult)
                nc.sync.dma_start(out=out[sys * D:sys * D + D, :],
                                  in_=t_pot[:])


    nc.compile()
    return nc


def _host_inputs(features, positions, cells, neighbor_indices,
                 neighbor_distances, W, b):
    features = np.asarray(features, np.float32)
    positions = np.asarray(positions, np.float32)
    cells = np.asarray(cells, np.float32)
    nidx = np.asarray(neighbor_indices)
    ndist = np.asarray(neighbor_distances, np.float32).reshape(S, E)
    W = np.asarray(W, np.float32)
    b = np.asarray(b, np.float32)

    assert np.allclose(cells, LCELL * np.eye(3, dtype=np.float32)[None]), \
        "kernel specialized to cubic L=8 cells"

    nh = _half_kgrid()
    K = len(nh)
    ksq = (2.0 * PI / LCELL) ** 2 * (nh * nh).sum(1).astype(np.float64)
    vol = LCELL ** 3
    G = 2.0 * PREF * (4.0 * PI / ksq) * np.exp(-0.5 * SMEAR**2 * ksq) / vol
    KT0 = (K + 127) // 128
    Gpad = np.zeros(KT0 * 128, np.float64)
    Gpad[:K] = G
    Gcol = Gpad.reshape(KT0, 128).T.astype(np.float32).copy()  # [128, KT0]

    layers_d, layers_i, Rs = _sr_arrange(nidx, ndist)

    # per-core input maps
    nt3 = nh.T.astype(np.float16)          # [3, K]
    nt6 = np.concatenate([nt3, nt3], 0)    # [6, K]
    WT_aug = np.concatenate([W.T, b[None, :]], 0).astype(np.float32)  # [65, 64]
    negI = (-np.eye(128)).astype(np.float16)
    id16 = np.eye(128).astype(np.float16)
    id32 = np.eye(128).astype(np.float32)

    in_maps = []
    for core in range(NCORES):
        s0 = core * SYS_PER_CORE
        fa = []
        p6 = []
        for s in range(s0, s0 + SYS_PER_CORE):
            f = features[s * N:(s + 1) * N].T                      # [64, 512]
            fa.append(np.concatenate([f, np.ones((1, N), np.float32)], 0))
            pf = (positions[s].T.astype(np.float64)) / LCELL       # [3, 512]
            ph = pf.astype(np.float16)
            pl = (pf - ph.astype(np.float64)).astype(np.float16)
            p6.append(np.concatenate([ph, pl], 0))                 # [6, 512]
        m = {
            "G16row": np.broadcast_to(G.astype(np.float16)[None, :], (64, len(G))).copy(),
            "featT": np.concatenate(fa, 1),
            "pT6": np.concatenate(p6, 1),
            "WT": WT_aug,
            "nt6": nt6,
            "Gcol": Gcol,
            "negI": negI,
            "id16": id16,
            "id32": id32,
        }
        for l in range(len(Rs)):
            R = Rs[l]
            dd = layers_d[l][s0:s0 + SYS_PER_CORE].reshape(-1, R)  # [1024, R]
            ii = layers_i[l][s0:s0 + SYS_PER_CORE].reshape(-1, R)
            m[f"srd{l}"] = np.concatenate(
                [dd[b * 128:(b + 1) * 128] for b in range(SYS_PER_CORE * 4)], 1)
            m[f"sri{l}"] = np.concatenate(
                [ii[b * 128:(b + 1) * 128] for b in range(SYS_PER_CORE * 4)], 1)
        in_maps.append(m)
    return in_maps, K, tuple(Rs)


def kernel(features, positions, cells, neighbor_indices, neighbor_distances,
           W, b, _trace=False):
    in_maps, K, Rs = _host_inputs(features, positions, cells, neighbor_indices,
                                  neighbor_distances, W, b)
    key = (K, Rs)
    if key not in _CACHE:
        _CACHE[key] = _build_nc(K, list(Rs))
    nc = _CACHE[key]
    res = bass_utils.run_bass_kernel_spmd(nc, in_maps,
                                          core_ids=list(range(NCORES)),
                                          trace=_trace)
    blocks = []
    for i in range(NCORES):
        o = res.results[i]["out"]  # [SC*D, N] transposed per system
        for sys in range(SYS_PER_CORE):
            blocks.append(o[sys * D:(sys + 1) * D, :].T)
    out = np.concatenate(blocks, 0)
    if _trace:
        kernel.last_result = res
    return np.ascontiguousarray(out, dtype=np.float32)


def measure_hw_ns(features, positions, cells, neighbor_indices,
                  neighbor_distances, W, b, reps=300):
    """Time the kernel on hardware via an on-device repeat loop (amortizes
    the multi-ms axon RPC dispatch overhead). Returns per-iteration ns."""
    import time
    import jax
    from jax.sharding import Mesh, PartitionSpec, NamedSharding
    from jax.experimental.shard_map import shard_map
    from concourse import bass2jax
    from concourse.bass2jax import _bass_exec_p, partition_id_tensor

    bass2jax.install_neuronx_cc_hook()
    in_maps, K, Rs = _host_inputs(features, positions, cells, neighbor_indices,
                                  neighbor_distances, W, b)

    def build_fn(nc, mesh, sh):
        partition_name = (nc.partition_id_tensor.name
                          if nc.partition_id_tensor else None)
        in_names, out_names, out_avals, zero_outs = [], [], [], []
        for alloc in nc.m.functions[0].allocations:
            if not isinstance(alloc, mybir.MemoryLocationSet):
                continue
            name = alloc.memorylocations[0].name
            if alloc.kind == "ExternalInput":
                if name != partition_name:
                    in_names.append(name)
            elif alloc.kind == "ExternalOutput":
                shape = tuple(alloc.tensor_shape)
                dtype = mybir.dt.np(alloc.dtype)
                out_names.append(name)
                out_avals.append(jax.core.ShapedArray(shape, dtype))
                zero_outs.append(np.zeros(shape, dtype))
        n_params = len(in_names)
        all_names = in_names + out_names
        if partition_name is not None:
            all_names = all_names + [partition_name]

        def _body(*args):
            operands = list(args)
            if partition_name is not None:
                operands.append(partition_id_tensor())
            return tuple(_bass_exec_p.bind(
                *operands, out_avals=tuple(out_avals), in_names=tuple(all_names),
                out_names=tuple(out_names), lowering_input_output_aliases=(),
                sim_require_finite=True, sim_require_nnan=True, nc=nc))

        specs_in = (PartitionSpec("core"),) * (n_params + len(out_names))
        specs_out = (PartitionSpec("core"),) * len(out_names)
        fn = jax.jit(shard_map(_body, mesh=mesh, in_specs=specs_in,
                               out_specs=specs_out, check_rep=False),
                     keep_unused=True)
        cat = [np.concatenate([np.asarray(in_maps[c][in_names[i]])
                               for c in range(NCORES)], 0)
               for i in range(n_params)]
        cat += [np.zeros((NCORES * z.shape[0], *z.shape[1:]), z.dtype)
                for z in zero_outs]
        dev = [jax.device_put(a, sh) for a in cat]
        return fn, dev

    devices = jax.devices()[:NCORES]
    mesh = Mesh(np.asarray(devices), ("core",))
    sh = NamedSharding(mesh, PartitionSpec("core"))

    def time_min(fn, dev, n=8):
        o = fn(*dev); jax.block_until_ready(o)
        best = float("inf")
        for _ in range(n):
            t0 = time.perf_counter()
            o = fn(*dev); jax.block_until_ready(o)
            best = min(best, (time.perf_counter() - t0) * 1e9)
        return best

    key1 = (K, Rs)
    if key1 not in _CACHE:
        _CACHE[key1] = _build_nc(K, list(Rs))
    fn1, dev1 = build_fn(_CACHE[key1], mesh, sh)
    t1 = time_min(fn1, dev1)
    keyr = (K, Rs, reps)
    if keyr not in _CACHE:
        _CACHE[keyr] = _build_nc(K, list(Rs), reps=reps)
    fnr, devr = build_fn(_CACHE[keyr], mesh, sh)
    tr = time_min(fnr, devr)
    return (tr - t1) / (reps - 1)



# revision 10
# speedup vs baseline: 3.4485x; 3.4485x over previous
"""Trainium2 Bass kernel for nn_LongRangeFeaturizer (Ewald sum featurizer).

Shards the 16 independent systems across 8 NeuronCores (2 systems/core).
All heavy math (charges matmul, k-space structure factors, trig, short-range
erf/cutoff coefficients, scatter, final combine) runs on-device.

Key structure (v2):
 - k-grid truncated to |n|^2 <= 24 (242 half-grid vectors, padded to 256);
   the dropped shells contribute < 2e-4 relative error (G ~ exp(-k^2/2)/k^2).
 - trig computed once in [K, 2N] layout; the [N, K] layout for stage 1 is
   produced by PE transposes instead of a second trig pass.
 - Ewald self term folded into the short-range scatter matrix as diagonal
   edges with d ~ 0: sr(d->0) = -sqrt(2/pi)/sigma exactly.
 - background (k=0) term folded into a padded k slot with G = -pi*sigma^2/V.
   The final combine is then a single multiply: out = pot * q.
 - short-range scatter uses 1024-wide outputs: layer-0 edge (j,i) -> col i,
   duplicate (j,i) -> col 512+i; both halves are separate matmul rhs. Third
   occurrences (a handful) go to one row-compacted [128,1024] matrix whose
   lhsT is a host-gathered feature matmul.
"""

import sys

sys.path.insert(0, "/opt/trn_rl_repo")

import numpy as np

import concourse.bass as bass
import concourse.mybir as mybir
import concourse.tile as tile
from concourse import bacc, bass_utils

dt = mybir.dt
F32, F16, I16 = dt.float32, dt.float16, dt.int16
AF = mybir.ActivationFunctionType
AOP = mybir.AluOpType

PI = float(np.pi)
MAGIC = float(1.5 * 2**23)  # round-to-nearest-int magic constant for fp32

# Problem constants
S, N, D, E = 16, 512, 64, 16384
LCELL = 8.0
SMEAR = 1.0
EXCL = 5.0
LRWL = 1.0
PREF = 1.0
NMAX = 8
NCORES = 8
SYS_PER_CORE = S // NCORES

NSQ_CUT = 24          # keep |n|^2 <= 24; truncation err ~1.5e-4 rel
K2 = 256              # padded half-grid size (2 k-tiles)
DIAG_DIST = 0.01      # sr(0.01) ~= -sqrt(2/pi) = -selfc
PAD_DIST = float(EXCL)  # fcut(EXCL) = 0 -> padded slots contribute ~0

_CACHE = {}


def _half_kgrid():
    r = np.arange(-NMAX, NMAX + 1)
    n = np.stack(np.meshgrid(r, r, r, indexing="ij"), -1).reshape(-1, 3)
    n = n[np.any(n != 0, axis=1)]
    nsq = (n * n).sum(1)
    n = n[nsq <= NSQ_CUT]
    pos = (n[:, 0] > 0) | ((n[:, 0] == 0) & (n[:, 1] > 0)) | (
        (n[:, 0] == 0) & (n[:, 1] == 0) & (n[:, 2] > 0)
    )
    return n[pos].astype(np.int64)  # [K0, 3]


def _sr_arrange(nidx, ndist):
    """Per-system edge layering with appended diagonal (self-term) edges.

    Returns per-system lists of (l0, l1, l2) edge arrays (j, i, d) and the
    global widths R0, R1, W3."""
    per_sys = []
    R0 = R1 = W3 = 0
    for s in range(S):
        j_t = np.concatenate([nidx[s, :, 1].astype(np.int64), np.arange(N)])
        i_t = np.concatenate([nidx[s, :, 0].astype(np.int64), np.arange(N)])
        d_t = np.concatenate([ndist[s].astype(np.float64),
                              np.full(N, DIAG_DIST)])
        cid = j_t * N + i_t
        order = np.argsort(cid, kind="stable")
        cs, js, is_, ds_ = cid[order], j_t[order], i_t[order], d_t[order]
        first = np.concatenate([[0], np.nonzero(np.diff(cs))[0] + 1])
        run_id = np.zeros(len(cs), np.int64)
        run_id[first] = 1
        run_id = np.cumsum(run_id) - 1
        occ = np.arange(len(cs)) - first[run_id]
        layers = []
        for lay, sel in ((0, occ == 0), (1, occ == 1), (2, occ >= 2)):
            layers.append((js[sel], is_[sel], ds_[sel]))
        per_sys.append(layers)
        c0 = np.bincount(layers[0][0], minlength=N).max()
        c1 = np.bincount(layers[1][0], minlength=N).max() if len(layers[1][0]) else 0
        R0 = max(R0, int(c0))
        R1 = max(R1, int(c1))
    # W3: max l2 slots needed in any compact row (per (sys, j) row)
    for s in range(S):
        js2 = per_sys[s][2][0]
        if len(js2):
            W3 = max(W3, int(np.bincount(js2).max()))
    R01 = R0 + R1
    R01 += R01 % 2
    W3 = max(2, W3 + (W3 % 2))
    return per_sys, R01, R0, W3


def _build_nc(R01, W3, reps=1):
    """Build the per-core SPMD program."""
    nc = bacc.Bacc("TRN2", target_bir_lowering=False, debug=False,
                   num_devices=NCORES)

    # const APs for activation biases
    for val in (PI / 2,):
        t = nc.alloc_sbuf_tensor(f"constap-{val}", [128, 1], F32)
        nc.gpsimd.memset(t.ap(), val)
        nc.const_aps.aps[(F32, val)] = t.ap()
    nc.all_engine_barrier()

    def din(name, shape, d=F16):
        return nc.dram_tensor(name, shape, d, kind="ExternalInput").ap()

    SC = SYS_PER_CORE
    featT = din("featT", [D + 1, SC * N])     # [65, 1024] f16 (features.T ; 1)
    featS = din("featS", [D + 1, 128])        # f16 sigma-gathered features
    pT6 = din("pT6", [6, SC * N])             # f16 hi/lo frac positions
    WT = din("WT", [D + 1, D])                # f16 (W.T ; b)
    nt6 = din("nt6", [6, K2])                 # f16 [n;n]
    G2 = din("G2", [128, 2 * K2])             # f16 [G|G] broadcast rows
    negI = din("negI", [128, 128])            # f16 -I
    id16 = din("id16", [128, 128])            # f16 I
    WSR = 8 * R01 + W3
    srd = din("srd", [128, WSR])              # f16 slot distances
    sri = din("sri", [128, WSR], I16)         # i16 slot column indices
    out = nc.dram_tensor("out", [SC * D, N], F32, kind="ExternalOutput").ap()

    NT = N // 128   # 4 atom tiles per system
    KT = K2 // 128  # 2 k tiles

    from contextlib import nullcontext
    with tile.TileContext(nc) as tc:
        with (
            tc.tile_pool(name="const", bufs=1) as cp,
            tc.tile_pool(name="work", bufs=2) as wp,
            tc.tile_pool(name="keep", bufs=1) as tp,
            tc.tile_pool(name="psU", bufs=2, space="PSUM") as pU,
            tc.tile_pool(name="psT", bufs=2, space="PSUM") as pT,
            tc.tile_pool(name="psH", bufs=1, space="PSUM") as pH,
            tc.For_i(0, reps, 1) if reps > 1 else nullcontext(),
        ):
            # ---- input DMAs ----
            t_srd = cp.tile([128, WSR], F16, tag="srd")
            nc.sync.dma_start(out=t_srd[:], in_=srd[:])
            t_sri = cp.tile([128, WSR], I16, tag="sri")
            nc.sync.dma_start(out=t_sri[:], in_=sri[:])
            t_feat = cp.tile([D + 1, SC * N], F16, tag="feat")
            nc.sync.dma_start(out=t_feat[:], in_=featT[:])
            t_featS = cp.tile([D + 1, 128], F16, tag="featS")
            nc.sync.dma_start(out=t_featS[:], in_=featS[:])
            t_pT6 = cp.tile([6, SC * N], F16, tag="p6")
            nc.sync.dma_start(out=t_pT6[:], in_=pT6[:])
            t_WT = cp.tile([D + 1, D], F16, tag="wt")
            nc.sync.dma_start(out=t_WT[:], in_=WT[:])
            t_nt6 = cp.tile([6, K2], F16, tag="nt6")
            nc.sync.dma_start(out=t_nt6[:], in_=nt6[:])
            t_G2 = cp.tile([128, 2 * K2], F16, tag="g2")
            nc.sync.dma_start(out=t_G2[:], in_=G2[:])
            t_negI = cp.tile([128, 128], F16, tag="negi")
            nc.sync.dma_start(out=t_negI[:], in_=negI[:])
            t_id16 = cp.tile([128, 128], F16, tag="id16")
            nc.sync.dma_start(out=t_id16[:], in_=id16[:])

            # ---- short-range coefficients (fp16 pipeline) ----
            # sr(d) = (erf(d/sqrt2) * (1/d)) * (-0.5 - 0.5*sin(pi*d/5 + pi/2))
            t_erf = wp.tile([128, WSR], F16, tag="srerf")
            nc.scalar.activation(t_erf[:], t_srd[:], AF.Erf,
                                 scale=float(1 / np.sqrt(2.0)))
            t_fc = wp.tile([128, WSR], F16, tag="srfc")
            nc.scalar.activation(t_fc[:], t_srd[:], AF.Sin,
                                 scale=float(PI / EXCL), bias=PI / 2)
            t_rec = wp.tile([128, WSR], F16, tag="srrec")
            with nc.allow_low_precision(reason="fp16 sr coefficients, 2e-2 tol"):
                nc.vector.reciprocal(t_rec[:], t_srd[:])
            t_fc2 = wp.tile([128, WSR], F16, tag="srfc2")
            nc.vector.tensor_scalar(out=t_fc2[:], in0=t_fc[:],
                                    scalar1=-0.5 * PREF, scalar2=-0.5 * PREF,
                                    op0=AOP.mult, op1=AOP.add)
            t_m1 = wp.tile([128, WSR], F16, tag="srm1")
            nc.vector.tensor_tensor(out=t_m1[:], in0=t_erf[:], in1=t_rec[:],
                                    op=AOP.mult)
            t_sr = wp.tile([128, WSR], F16, tag="srv")
            nc.vector.tensor_tensor(out=t_sr[:], in0=t_m1[:], in1=t_fc2[:],
                                    op=AOP.mult)

            # ---- scatters: M2[blk] [128, 1024] (l0 cols 0-511, l1 512-1023);
            #      M3 [128, 1024] row-compacted third occurrences ----
            t_M2 = []
            for blk in range(8):
                m = tp.tile([128, 2 * N], F16, tag=f"m2_{blk}")
                csl = slice(blk * R01, (blk + 1) * R01)
                nc.gpsimd.local_scatter(out_ap=m[:], data_ap=t_sr[:, csl],
                                        idxs_ap=t_sri[:, csl], channels=128,
                                        num_elems=2 * N, num_idxs=R01)
                t_M2.append(m)
            t_M3 = tp.tile([128, 2 * N], F16, tag="m3")
            csl = slice(8 * R01, 8 * R01 + W3)
            nc.gpsimd.local_scatter(out_ap=t_M3[:], data_ap=t_sr[:, csl],
                                    idxs_ap=t_sri[:, csl], channels=128,
                                    num_elems=2 * N, num_idxs=W3)

            # ---- charges ----
            # qT (f32, psum-resident until the final combine):
            # rows [sys*64, sys*64+64) = q.T of system sys
            ps_qT = pH.tile([128, N], F32, tag="qt")
            for sys in range(SC):
                csl = slice(sys * N, sys * N + N)
                nc.tensor.matmul(out=ps_qT[sys * D:(sys + 1) * D, :],
                                 lhsT=t_WT[:], rhs=t_feat[:, csl],
                                 start=True, stop=True)
            t_qT = tp.tile([128, N], F32, tag="qtf")
            nc.scalar.activation(t_qT[:], ps_qT[:], AF.Copy)
            # q16 tiles [128(n), 64] per (sys, nt)
            t_q16 = [[None] * NT for _ in range(SC)]
            for sys in range(SC):
                for nt_i in range(NT):
                    fsl = slice(sys * N + nt_i * 128, sys * N + nt_i * 128 + 128)
                    ps_q = pU.tile([128, N], F32, tag="uc")
                    nc.tensor.matmul(out=ps_q[:, 0:D], lhsT=t_feat[:, fsl],
                                     rhs=t_WT[:], start=True, stop=True)
                    tq = tp.tile([128, D], F16, tag=f"q16_{sys}_{nt_i}")
                    nc.scalar.activation(tq[:], ps_q[:, 0:D], AF.Copy)
                    t_q16[sys][nt_i] = tq
            # q_sigma for the l2-compact matmul
            ps_qs = pU.tile([128, N], F32, tag="uc")
            nc.tensor.matmul(out=ps_qs[:, 0:D], lhsT=t_featS[:], rhs=t_WT[:],
                             start=True, stop=True)
            t_qs = tp.tile([128, D], F16, tag="qsig")
            nc.scalar.activation(t_qs[:], ps_qs[:, 0:D], AF.Copy)

            # ---- trig in KN layout: c,s [K2, 2N] f16, chunked 512 ----
            t_c = [tp.tile([128, SC * N], F16, tag=f"ckn{kt}",
                           name=f"ckn{kt}") for kt in range(KT)]
            t_s = [tp.tile([128, SC * N], F16, tag=f"skn{kt}",
                           name=f"skn{kt}") for kt in range(KT)]
            for kt in range(KT):
                ksl = slice(kt * 128, kt * 128 + 128)
                for h in range(SC):
                    hsl = slice(h * N, h * N + N)
                    ps_u = pU.tile([128, N], F32, tag="uc")
                    nc.tensor.matmul(out=ps_u[:], lhsT=t_nt6[:, ksl],
                                     rhs=t_pT6[:, hsl], start=True, stop=False)
                    t_r = wp.tile([128, N], F16, tag="rnd")
                    nc.vector.tensor_scalar(out=t_r[:], in0=ps_u[:],
                                            scalar1=MAGIC, scalar2=MAGIC,
                                            op0=AOP.add, op1=AOP.subtract)
                    nc.tensor.matmul(out=ps_u[:], lhsT=t_negI[:], rhs=t_r[:],
                                     start=False, stop=True)
                    nc.scalar.activation(t_s[kt][:, hsl], ps_u[:], AF.Sin,
                                         scale=2 * PI)
                    t_a = wp.tile([128, N], F32, tag="abs")
                    nc.scalar.activation(t_a[:], ps_u[:], AF.Abs)
                    nc.scalar.activation(t_c[kt][:, hsl], t_a[:], AF.Sin,
                                         scale=-2 * PI, bias=PI / 2)

            # ---- NK layout via PE transposes: per (sys, nt) [128(n), 512]
            #      cols = [c_kt0 | c_kt1 | s_kt0 | s_kt1] ----
            t_cs = [[None] * NT for _ in range(SC)]
            for sys in range(SC):
                for nt_i in range(NT):
                    nsl = slice(sys * N + nt_i * 128, sys * N + nt_i * 128 + 128)
                    ps_tr = pT.tile([128, 512], F16, tag="tr")
                    for q, src in enumerate((t_c[0], t_c[1], t_s[0], t_s[1])):
                        nc.tensor.transpose(out=ps_tr[:, q * 128:(q + 1) * 128],
                                            in_=src[:, nsl],
                                            identity=t_id16[:])
                    tt_ = tp.tile([128, 512], F16, tag=f"cs{sys}_{nt_i}")
                    nc.vector.tensor_copy(out=tt_[:], in_=ps_tr[:])
                    t_cs[sys][nt_i] = tt_

            # ---- stage1: [ScT | SsT] [64, 512] per sys in one psum ----
            ps_S = pH.tile([128, 2 * K2], F32, tag="s1")
            for sys in range(SC):
                for nt_i in range(NT):
                    nc.tensor.matmul(out=ps_S[sys * D:(sys + 1) * D, :],
                                     lhsT=t_q16[sys][nt_i][:],
                                     rhs=t_cs[sys][nt_i][:],
                                     start=(nt_i == 0), stop=(nt_i == NT - 1))
            # multiply by G (both systems in one op)
            t_GS = wp.tile([128, 2 * K2], F16, tag="gs")
            nc.vector.tensor_tensor(out=t_GS[:], in0=ps_S[:], in1=t_G2[:],
                                    op=AOP.mult)

            # ---- transpose GS -> [K, 64] blocks: [c0 | c1 | s0 | s1] ----
            t_GST = []
            for sys in range(SC):
                ps_g = pT.tile([128, 512], F16, tag="tr")
                for b in range(4):
                    nc.tensor.transpose(
                        out=ps_g[:, b * D:(b + 1) * D],
                        in_=t_GS[sys * D:(sys + 1) * D, b * 128:(b + 1) * 128],
                        identity=t_id16[sys * D:(sys + 1) * D,
                                        sys * D:(sys + 1) * D])
                tg = tp.tile([128, 4 * D], F16, tag=f"gst{sys}")
                nc.vector.tensor_copy(out=tg[:], in_=ps_g[:, 0:4 * D])
                t_GST.append(tg)

            # ---- stage2 + M@q + l2 into one psum accumulation ----
            ps_pot = pH.tile([128, N], F32, tag="pot")
            for sys in range(SC):
                csl = slice(sys * N, sys * N + N)
                orow = slice(sys * D, (sys + 1) * D)
                for b in range(4):
                    src = t_c[b % 2] if b < 2 else t_s[b % 2]
                    nc.tensor.matmul(out=ps_pot[orow, :],
                                     lhsT=t_GST[sys][:, b * D:(b + 1) * D],
                                     rhs=src[:, csl], start=(b == 0),
                                     stop=False)
                for jt in range(NT):
                    m = t_M2[sys * NT + jt]
                    nc.tensor.matmul(out=ps_pot[orow, :],
                                     lhsT=t_q16[sys][jt][:], rhs=m[:, 0:N],
                                     start=False, stop=False)
                    nc.tensor.matmul(out=ps_pot[orow, :],
                                     lhsT=t_q16[sys][jt][:], rhs=m[:, N:2 * N],
                                     start=False, stop=False)
                nc.tensor.matmul(out=ps_pot[orow, :], lhsT=t_qs[:],
                                 rhs=t_M3[:, csl], start=False, stop=True)

            # ---- combine: out = pot * q ----
            t_out = wp.tile([128, N], F32, tag="outf")
            nc.vector.tensor_tensor(out=t_out[:], in0=ps_pot[:], in1=t_qT[:],
                                    op=AOP.mult)
            nc.sync.dma_start(out=out[:], in_=t_out[:])

    nc.compile()
    return nc


def _host_inputs(features, positions, cells, neighbor_indices,
                 neighbor_distances, W, b):
    features = np.asarray(features, np.float32)
    positions = np.asarray(positions, np.float32)
    cells = np.asarray(cells, np.float32)
    nidx = np.asarray(neighbor_indices)
    ndist = np.asarray(neighbor_distances, np.float32).reshape(S, E)
    W = np.asarray(W, np.float32)
    b = np.asarray(b, np.float32)

    assert np.allclose(cells, LCELL * np.eye(3, dtype=np.float32)[None]), \
        "kernel specialized to cubic L=8 cells"

    nh = _half_kgrid()
    K0 = len(nh)
    assert K0 <= K2 - 1
    ksq = (2.0 * PI / LCELL) ** 2 * (nh * nh).sum(1).astype(np.float64)
    vol = LCELL ** 3
    bgov = PREF * float(PI * SMEAR**2 / vol)
    G = 2.0 * PREF * (4.0 * PI / ksq) * np.exp(-0.5 * SMEAR**2 * ksq) / vol
    Gpad = np.zeros(K2, np.float64)
    Gpad[:K0] = G
    Gpad[K0] = -bgov  # background term via the k=0 pad slot (c=1, s=0)
    G2row = np.concatenate([Gpad, Gpad]).astype(np.float16)
    G2 = np.broadcast_to(G2row[None, :], (128, 2 * K2)).copy()

    per_sys, R01, R0, W3 = _sr_arrange(nidx, ndist)

    nt3 = np.zeros((3, K2), np.float16)
    nt3[:, :K0] = nh.T.astype(np.float16)
    nt6 = np.concatenate([nt3, nt3], 0)    # [6, K2]
    WT_aug = np.concatenate([W.T, b[None, :]], 0).astype(np.float16)  # [65, 64]
    negI = (-np.eye(128)).astype(np.float16)
    id16 = np.eye(128).astype(np.float16)

    WSR = 8 * R01 + W3
    in_maps = []
    for core in range(NCORES):
        s0 = core * SYS_PER_CORE
        fa = []
        p6 = []
        for s in range(s0, s0 + SYS_PER_CORE):
            f = features[s * N:(s + 1) * N].T                      # [64, 512]
            fa.append(np.concatenate([f, np.ones((1, N), np.float32)], 0))
            pf = (positions[s].T.astype(np.float64)) / LCELL       # [3, 512]
            ph = pf.astype(np.float16)
            pl = (pf - ph.astype(np.float64)).astype(np.float16)
            p6.append(np.concatenate([ph, pl], 0))                 # [6, 512]

        srd_c = np.full((128, WSR), PAD_DIST, np.float16)
        sri_c = np.full((128, WSR), -1, np.int16)
        sig_rows = []   # (sys_local, j) for l2-compact rows
        for sys_local in range(SYS_PER_CORE):
            l0, l1, l2 = per_sys[s0 + sys_local]
            for lay, (js, is_, ds_) in enumerate((l0, l1)):
                if len(js) == 0:
                    continue
                cnt = np.bincount(js, minlength=N)
                start = np.concatenate([[0], np.cumsum(cnt)[:-1]])
                slot = np.arange(len(js)) - start[js]
                blk = sys_local * 4 + js // 128
                row = js % 128
                base = 0 if lay == 0 else R0
                col = blk * R01 + base + slot
                srd_c[row, col] = ds_.astype(np.float16)
                sri_c[row, col] = (is_ + (0 if lay == 0 else N)).astype(np.int16)
            js, is_, ds_ = l2
            if len(js):
                uj = np.unique(js)
                rowmap = {}
                for j in uj:
                    rowmap[j] = len(sig_rows)
                    sig_rows.append((sys_local, int(j)))
                used = {}
                for j, i, d in zip(js, is_, ds_):
                    p = rowmap[j]
                    t = used.get(p, 0)
                    used[p] = t + 1
                    srd_c[p, 8 * R01 + t] = np.float16(d)
                    sri_c[p, 8 * R01 + t] = np.int16(sys_local * N + i)
        assert len(sig_rows) <= 128, f"l2 rows {len(sig_rows)} > 128"
        featS = np.zeros((D + 1, 128), np.float16)
        for p, (sys_local, j) in enumerate(sig_rows):
            s = s0 + sys_local
            featS[:D, p] = features[s * N + j].astype(np.float16)
            featS[D, p] = 1.0

        m = {
            "featT": np.concatenate(fa, 1).astype(np.float16),
            "featS": featS,
            "pT6": np.concatenate(p6, 1),
            "WT": WT_aug,
            "nt6": nt6,
            "G2": G2,
            "negI": negI,
            "id16": id16,
            "srd": srd_c,
            "sri": sri_c,
        }
        in_maps.append(m)
    return in_maps, R01, W3


def kernel(features, positions, cells, neighbor_indices, neighbor_distances,
           W, b, _trace=False):
    in_maps, R01, W3 = _host_inputs(features, positions, cells,
                                    neighbor_indices, neighbor_distances, W, b)
    key = (R01, W3)
    if key not in _CACHE:
        _CACHE[key] = _build_nc(R01, W3)
    nc = _CACHE[key]
    res = bass_utils.run_bass_kernel_spmd(nc, in_maps,
                                          core_ids=list(range(NCORES)),
                                          trace=_trace)
    blocks = []
    for i in range(NCORES):
        o = res.results[i]["out"]  # [SC*D, N] transposed per system
        for sys in range(SYS_PER_CORE):
            blocks.append(o[sys * D:(sys + 1) * D, :].T)
    out = np.concatenate(blocks, 0)
    if _trace:
        kernel.last_result = res
    return np.ascontiguousarray(out, dtype=np.float32)


def measure_hw_ns(features, positions, cells, neighbor_indices,
                  neighbor_distances, W, b, reps=300):
    """Time the kernel on hardware via an on-device repeat loop (amortizes
    the multi-ms axon RPC dispatch overhead). Returns per-iteration ns."""
    import time
    import jax
    from jax.sharding import Mesh, PartitionSpec, NamedSharding
    from jax.experimental.shard_map import shard_map
    from concourse import bass2jax
    from concourse.bass2jax import _bass_exec_p, partition_id_tensor

    bass2jax.install_neuronx_cc_hook()
    in_maps, R01, W3 = _host_inputs(features, positions, cells,
                                    neighbor_indices, neighbor_distances, W, b)

    def build_fn(nc, mesh, sh):
        partition_name = (nc.partition_id_tensor.name
                          if nc.partition_id_tensor else None)
        in_names, out_names, out_avals, zero_outs = [], [], [], []
        for alloc in nc.m.functions[0].allocations:
            if not isinstance(alloc, mybir.MemoryLocationSet):
                continue
            name = alloc.memorylocations[0].name
            if alloc.kind == "ExternalInput":
                if name != partition_name:
                    in_names.append(name)
            elif alloc.kind == "ExternalOutput":
                shape = tuple(alloc.tensor_shape)
                dtype = mybir.dt.np(alloc.dtype)
                out_names.append(name)
                out_avals.append(jax.core.ShapedArray(shape, dtype))
                zero_outs.append(np.zeros(shape, dtype))
        n_params = len(in_names)
        all_names = in_names + out_names
        if partition_name is not None:
            all_names = all_names + [partition_name]

        def _body(*args):
            operands = list(args)
            if partition_name is not None:
                operands.append(partition_id_tensor())
            return tuple(_bass_exec_p.bind(
                *operands, out_avals=tuple(out_avals), in_names=tuple(all_names),
                out_names=tuple(out_names), lowering_input_output_aliases=(),
                sim_require_finite=True, sim_require_nnan=True, nc=nc))

        specs_in = (PartitionSpec("core"),) * (n_params + len(out_names))
        specs_out = (PartitionSpec("core"),) * len(out_names)
        fn = jax.jit(shard_map(_body, mesh=mesh, in_specs=specs_in,
                               out_specs=specs_out, check_rep=False),
                     keep_unused=True)
        cat = [np.concatenate([np.asarray(in_maps[c][in_names[i]])
                               for c in range(NCORES)], 0)
               for i in range(n_params)]
        cat += [np.zeros((NCORES * z.shape[0], *z.shape[1:]), z.dtype)
                for z in zero_outs]
        dev = [jax.device_put(a, sh) for a in cat]
        return fn, dev

    devices = jax.devices()[:NCORES]
    mesh = Mesh(np.asarray(devices), ("core",))
    sh = NamedSharding(mesh, PartitionSpec("core"))

    def time_min(fn, dev, n=8):
        o = fn(*dev); jax.block_until_ready(o)
        best = float("inf")
        for _ in range(n):
            t0 = time.perf_counter()
            o = fn(*dev); jax.block_until_ready(o)
            best = min(best, (time.perf_counter() - t0) * 1e9)
        return best

    key1 = (R01, W3)
    if key1 not in _CACHE:
        _CACHE[key1] = _build_nc(R01, W3)
    fn1, dev1 = build_fn(_CACHE[key1], mesh, sh)
    t1 = time_min(fn1, dev1)
    keyr = (R01, W3, reps)
    if keyr not in _CACHE:
        _CACHE[keyr] = _build_nc(R01, W3, reps=reps)
    fnr, devr = build_fn(_CACHE[keyr], mesh, sh)
    tr = time_min(fnr, devr)
    return (tr - t1) / (reps - 1)


# revision 14
# speedup vs baseline: 4.0451x; 1.1730x over previous
"""Trainium2 Bass kernel for nn_LongRangeFeaturizer (Ewald sum featurizer).

Shards the 16 independent systems across 8 NeuronCores (2 systems/core).
All heavy math (charges matmul, k-space structure factors, trig, short-range
erf/cutoff coefficients, scatter, final combine) runs on-device.

Key structure (v3):
 - k-grid truncated to |n|^2 <= 24 (242 half-grid vectors, padded to 256);
   the dropped shells contribute < 2e-4 relative error (G ~ exp(-k^2/2)/k^2).
 - trig computed once in [K, 2N] layout; the [N, K] layout for stage 1 is
   produced by PE transposes instead of a second trig pass.
 - Ewald self term folded into the short-range scatter matrix as diagonal
   edges with d ~ 0: sr(d->0) = -sqrt(2/pi)/sigma exactly.
 - background (k=0) term folded into a padded k slot with G = -pi*sigma^2/V.
   The final combine is then a single multiply: out = pot * q.
 - short-range: first occurrence of each (j,i) edge goes through 512-wide
   local_scatter (8 calls, the only Pool work); duplicate occurrences
   (~530/system) are handled as 128-edge chunks: gather the edge's features
   (host-arranged columns), matmul to charges, scale rows by sr(d) in the
   PSUM->SBUF copy, then matmul against a host-built one-hot [slot -> i]
   matrix, accumulating straight into the potential PSUM.
"""

import sys

sys.path.insert(0, "/opt/trn_rl_repo")

import numpy as np

import concourse.bass as bass
import concourse.mybir as mybir
import concourse.tile as tile
from concourse import bacc, bass_utils

dt = mybir.dt
F32, F16, I16 = dt.float32, dt.float16, dt.int16
AF = mybir.ActivationFunctionType
AOP = mybir.AluOpType

PI = float(np.pi)
MAGIC = float(1.5 * 2**23)  # round-to-nearest-int magic constant for fp32

# Problem constants
S, N, D, E = 16, 512, 64, 16384
LCELL = 8.0
SMEAR = 1.0
EXCL = 5.0
LRWL = 1.0
PREF = 1.0
NMAX = 8
NCORES = 8
SYS_PER_CORE = S // NCORES

NSQ_CUT = 24          # keep |n|^2 <= 24; truncation err ~1.5e-4 rel
K2 = 256              # padded half-grid size (2 k-tiles)
DIAG_DIST = 0.01      # sr(0.01) ~= -sqrt(2/pi) = -selfc
PAD_DIST = float(EXCL)  # fcut(EXCL) = 0 -> padded slots contribute ~0

_CACHE = {}


def _half_kgrid():
    r = np.arange(-NMAX, NMAX + 1)
    n = np.stack(np.meshgrid(r, r, r, indexing="ij"), -1).reshape(-1, 3)
    n = n[np.any(n != 0, axis=1)]
    nsq = (n * n).sum(1)
    n = n[nsq <= NSQ_CUT]
    pos = (n[:, 0] > 0) | ((n[:, 0] == 0) & (n[:, 1] > 0)) | (
        (n[:, 0] == 0) & (n[:, 1] == 0) & (n[:, 2] > 0)
    )
    return n[pos].astype(np.int64)  # [K0, 3]


def _sr_arrange(nidx, ndist):
    """Per-system edge split: first occurrence of each (j,i) (incl. appended
    diagonal self edges) vs duplicates. Returns per-system (l0, l12) edge
    arrays and the widths R0 (l0 slots/row) and NCH (dup chunks/system)."""
    per_sys = []
    R0 = 0
    NCH = 1
    for s in range(S):
        j_t = np.concatenate([nidx[s, :, 1].astype(np.int64), np.arange(N)])
        i_t = np.concatenate([nidx[s, :, 0].astype(np.int64), np.arange(N)])
        d_t = np.concatenate([ndist[s].astype(np.float64),
                              np.full(N, DIAG_DIST)])
        cid = j_t * N + i_t
        order = np.argsort(cid, kind="stable")
        cs, js, is_, ds_ = cid[order], j_t[order], i_t[order], d_t[order]
        first = np.concatenate([[0], np.nonzero(np.diff(cs))[0] + 1])
        run_id = np.zeros(len(cs), np.int64)
        run_id[first] = 1
        run_id = np.cumsum(run_id) - 1
        occ = np.arange(len(cs)) - first[run_id]
        sel0 = occ == 0
        l0 = (js[sel0], is_[sel0], ds_[sel0])
        l12 = (js[~sel0], is_[~sel0], ds_[~sel0])
        per_sys.append((l0, l12))
        R0 = max(R0, int(np.bincount(l0[0], minlength=N).max()))
        NCH = max(NCH, -(-len(l12[0]) // 128))
    R0 += R0 % 2
    return per_sys, R0, NCH


def _build_nc(R0, NCH, reps=1):
    """Build the per-core SPMD program. NCH = dup chunks per system."""
    nc = bacc.Bacc("TRN2", target_bir_lowering=False, debug=False,
                   num_devices=NCORES)

    for val in (PI / 2,):
        t = nc.alloc_sbuf_tensor(f"constap-{val}", [128, 1], F32)
        nc.gpsimd.memset(t.ap(), val)
        nc.const_aps.aps[(F32, val)] = t.ap()
    nc.all_engine_barrier()

    def din(name, shape, d=F16):
        return nc.dram_tensor(name, shape, d, kind="ExternalInput").ap()

    SC = SYS_PER_CORE
    NCH2 = SC * NCH
    WSR = 8 * R0 + NCH2
    # misc pack columns: [G2 (512) | WT (64) | featC (NCH2*128)]
    MW = 2 * K2 + D + NCH2 * 128
    srd = din("srd", [128, WSR])              # f16 slot distances
    p6n6 = din("p6n6", [6, SC * N + K2])      # f16 [pT6 | nt6]
    sri = din("sri", [128, 8 * R0], I16)      # i16 l0 column indices
    nid = din("nid", [128, 256])              # f16 [-I | I]
    misc = din("misc", [128, MW])             # f16 [G2 | WT | featC]
    featT = din("featT", [D + 1, SC * N])     # f16 (features.T ; 1)
    oneh = din("oneh", [128, NCH2 * N])       # f16 one-hot slot->i
    out = nc.dram_tensor("out", [SC * D, N], F32, kind="ExternalOutput").ap()

    NT = N // 128   # 4 atom tiles per system
    KT = K2 // 128  # 2 k tiles

    from contextlib import nullcontext
    with tile.TileContext(nc) as tc:
        with (
            tc.tile_pool(name="const", bufs=1) as cp,
            tc.tile_pool(name="work", bufs=2) as wp,
            tc.tile_pool(name="keep", bufs=1) as tp,
            tc.tile_pool(name="psU", bufs=2, space="PSUM") as pU,
            tc.tile_pool(name="psT", bufs=2, space="PSUM") as pT,
            tc.tile_pool(name="psH", bufs=1, space="PSUM") as pH,
            tc.For_i(0, reps, 1) if reps > 1 else nullcontext(),
        ):
            # ---- input DMAs (order = HWDGE serialization order) ----
            t_srd = cp.tile([128, WSR], F16, tag="srd")
            nc.sync.dma_start(out=t_srd[:], in_=srd[:])
            t_p6n6 = cp.tile([6, SC * N + K2], F16, tag="p6")
            nc.sync.dma_start(out=t_p6n6[:], in_=p6n6[:])
            t_sri = cp.tile([128, 8 * R0], I16, tag="sri")
            nc.sync.dma_start(out=t_sri[:], in_=sri[:])
            t_nid = cp.tile([128, 256], F16, tag="nid")
            nc.sync.dma_start(out=t_nid[:], in_=nid[:])
            t_misc = cp.tile([128, MW], F16, tag="misc")
            nc.sync.dma_start(out=t_misc[:], in_=misc[:])
            t_feat = cp.tile([D + 1, SC * N], F16, tag="feat")
            nc.sync.dma_start(out=t_feat[:], in_=featT[:])
            t_oneh = cp.tile([128, NCH2 * N], F16, tag="oneh")
            nc.sync.dma_start(out=t_oneh[:], in_=oneh[:])

            t_pT6 = t_p6n6[:, 0:SC * N]
            t_nt6 = t_p6n6[:, SC * N:]
            t_negI = t_nid[:, 0:128]
            t_id16 = t_nid[:, 128:256]
            t_G2 = t_misc[:, 0:2 * K2]
            t_WT = t_misc[0:D + 1, 2 * K2:2 * K2 + D]
            fc_off = 2 * K2 + D

            # ---- Act table preload: Sin set via a dummy op, then Erf ----
            t_dum = wp.tile([128, 1], F16, tag="dum")
            nc.scalar.activation(t_dum[:], nc.const_aps.aps[(F32, PI / 2)],
                                 AF.Sin)

            # ---- short-range coefficients (fp16 pipeline) ----
            # sr(d) = (erf(d/sqrt2) * (1/d)) * (-0.5 - 0.5*sin(pi*d/5 + pi/2))
            t_erf = wp.tile([128, WSR], F16, tag="srerf")
            nc.scalar.activation(t_erf[:], t_srd[:], AF.Erf,
                                 scale=float(1 / np.sqrt(2.0)))
            t_fc = wp.tile([128, WSR], F16, tag="srfc")
            nc.scalar.activation(t_fc[:], t_srd[:], AF.Sin,
                                 scale=float(PI / EXCL), bias=PI / 2)
            t_rec = wp.tile([128, WSR], F16, tag="srrec")
            with nc.allow_low_precision(reason="fp16 sr coefficients, 2e-2 tol"):
                nc.vector.reciprocal(t_rec[:], t_srd[:])
            t_fc2 = wp.tile([128, WSR], F16, tag="srfc2")
            nc.vector.tensor_scalar(out=t_fc2[:], in0=t_fc[:],
                                    scalar1=-0.5 * PREF, scalar2=-0.5 * PREF,
                                    op0=AOP.mult, op1=AOP.add)
            t_m1 = wp.tile([128, WSR], F16, tag="srm1")
            nc.vector.tensor_tensor(out=t_m1[:], in0=t_erf[:], in1=t_rec[:],
                                    op=AOP.mult)
            t_sr = wp.tile([128, WSR], F16, tag="srv")
            nc.vector.tensor_tensor(out=t_sr[:], in0=t_m1[:], in1=t_fc2[:],
                                    op=AOP.mult)
            # fp32 copy of the dup-chunk sr columns (activation scale AP)
            t_srf = wp.tile([128, NCH2], F32, tag="srf")
            nc.vector.tensor_copy(out=t_srf[:], in_=t_sr[:, 8 * R0:])

            # ---- 8 l0 scatters: M[blk] [128, 512] ----
            t_M = []
            for blk in range(8):
                m = tp.tile([128, N], F16, tag=f"m_{blk}")
                csl = slice(blk * R0, (blk + 1) * R0)
                nc.gpsimd.local_scatter(out_ap=m[:], data_ap=t_sr[:, csl],
                                        idxs_ap=t_sri[:, csl], channels=128,
                                        num_elems=N, num_idxs=R0)
                t_M.append(m)

            # ---- trig in KN layout: c,s [K2, 2N] f16, chunked 512 ----
            t_c = [tp.tile([128, SC * N], F16, tag=f"ckn{kt}",
                           name=f"ckn{kt}") for kt in range(KT)]
            t_s = [tp.tile([128, SC * N], F16, tag=f"skn{kt}",
                           name=f"skn{kt}") for kt in range(KT)]
            for kt in range(KT):
                ksl = slice(SC * N + kt * 128, SC * N + kt * 128 + 128)
                for h in range(SC):
                    hsl = slice(h * N, h * N + N)
                    ps_u = pU.tile([128, N], F32, tag="uc")
                    nc.tensor.matmul(out=ps_u[:], lhsT=t_p6n6[:, ksl],
                                     rhs=t_pT6[:, hsl], start=True, stop=False)
                    t_r = wp.tile([128, N], F16, tag="rnd")
                    nc.vector.tensor_scalar(out=t_r[:], in0=ps_u[:],
                                            scalar1=MAGIC, scalar2=MAGIC,
                                            op0=AOP.add, op1=AOP.subtract)
                    nc.tensor.matmul(out=ps_u[:], lhsT=t_negI[:], rhs=t_r[:],
                                     start=False, stop=True)
                    nc.scalar.activation(t_s[kt][:, hsl], ps_u[:], AF.Sin,
                                         scale=2 * PI)
                    t_a = wp.tile([128, N], F32, tag="abs")
                    nc.scalar.activation(t_a[:], ps_u[:], AF.Abs)
                    nc.scalar.activation(t_c[kt][:, hsl], t_a[:], AF.Sin,
                                         scale=-2 * PI, bias=PI / 2)

            # ---- charges ----
            ps_qT = pH.tile([128, N], F32, tag="qt")
            for sys in range(SC):
                csl = slice(sys * N, sys * N + N)
                nc.tensor.matmul(out=ps_qT[sys * D:(sys + 1) * D, :],
                                 lhsT=t_WT[:], rhs=t_feat[:, csl],
                                 start=True, stop=True)
            t_qT = tp.tile([128, N], F32, tag="qtf")
            nc.vector.tensor_copy(out=t_qT[:], in_=ps_qT[:])
            t_q16 = [[None] * NT for _ in range(SC)]
            for sys in range(SC):
                for nt_i in range(NT):
                    fsl = slice(sys * N + nt_i * 128, sys * N + nt_i * 128 + 128)
                    ps_q = pU.tile([128, N], F32, tag="uc")
                    nc.tensor.matmul(out=ps_q[:, 0:D], lhsT=t_feat[:, fsl],
                                     rhs=t_WT[:], start=True, stop=True)
                    tq = tp.tile([128, D], F16, tag=f"q16_{sys}_{nt_i}")
                    nc.vector.tensor_copy(out=tq[:], in_=ps_q[:, 0:D])
                    t_q16[sys][nt_i] = tq
            # dup-chunk charges, scaled by sr(d) in the copy
            t_Y = []
            for gc in range(NCH2):
                ps_qc = pU.tile([128, N], F32, tag="uc")
                fsl = slice(fc_off + gc * 128, fc_off + (gc + 1) * 128)
                nc.tensor.matmul(out=ps_qc[:, 0:D], lhsT=t_misc[0:D + 1, fsl],
                                 rhs=t_WT[:], start=True, stop=True)
                ty = tp.tile([128, D], F16, tag=f"y{gc}", name=f"y{gc}")
                nc.scalar.activation(ty[:], ps_qc[:, 0:D], AF.Copy,
                                     scale=t_srf[:, gc:gc + 1])
                t_Y.append(ty)

            # ---- NK layout via PE transposes: per (sys, nt) [128(n), 512]
            #      cols = [c_kt0 | c_kt1 | s_kt0 | s_kt1] ----
            t_cs = [[None] * NT for _ in range(SC)]
            for sys in range(SC):
                for nt_i in range(NT):
                    nsl = slice(sys * N + nt_i * 128, sys * N + nt_i * 128 + 128)
                    ps_tr = pT.tile([128, 512], F16, tag="tr")
                    for q, src in enumerate((t_c[0], t_c[1], t_s[0], t_s[1])):
                        nc.tensor.transpose(out=ps_tr[:, q * 128:(q + 1) * 128],
                                            in_=src[:, nsl],
                                            identity=t_id16[:])
                    tt_ = tp.tile([128, 512], F16, tag=f"cs{sys}_{nt_i}")
                    nc.vector.tensor_copy(out=tt_[:], in_=ps_tr[:])
                    t_cs[sys][nt_i] = tt_

            # ---- stage1: [ScT | SsT] [64, 512] per sys in one psum ----
            ps_S = pH.tile([128, 2 * K2], F32, tag="s1")
            for sys in range(SC):
                for nt_i in range(NT):
                    nc.tensor.matmul(out=ps_S[sys * D:(sys + 1) * D, :],
                                     lhsT=t_q16[sys][nt_i][:],
                                     rhs=t_cs[sys][nt_i][:],
                                     start=(nt_i == 0), stop=(nt_i == NT - 1))
            t_GS = wp.tile([128, 2 * K2], F16, tag="gs")
            nc.vector.tensor_tensor(out=t_GS[:], in0=ps_S[:], in1=t_G2[:],
                                    op=AOP.mult)

            # ---- transpose GS -> [K, 64] blocks: [c0 | c1 | s0 | s1] ----
            t_GST = []
            for sys in range(SC):
                ps_g = pT.tile([128, 512], F16, tag="tr")
                for b in range(4):
                    nc.tensor.transpose(
                        out=ps_g[:, b * D:(b + 1) * D],
                        in_=t_GS[sys * D:(sys + 1) * D, b * 128:(b + 1) * 128],
                        identity=t_id16[sys * D:(sys + 1) * D,
                                        sys * D:(sys + 1) * D])
                tg = tp.tile([128, 4 * D], F16, tag=f"gst{sys}",
                             name=f"gst{sys}")
                nc.vector.tensor_copy(out=tg[:], in_=ps_g[:, 0:4 * D])
                t_GST.append(tg)

            # ---- stage2 + M@q + dup chunks into one psum; per-sys close ----
            ps_pot = pH.tile([128, N], F32, tag="pot")
            for sys in range(SC):
                csl = slice(sys * N, sys * N + N)
                orow = slice(sys * D, (sys + 1) * D)
                for b in range(4):
                    src = t_c[b % 2] if b < 2 else t_s[b % 2]
                    nc.tensor.matmul(out=ps_pot[orow, :],
                                     lhsT=t_GST[sys][:, b * D:(b + 1) * D],
                                     rhs=src[:, csl], start=(b == 0),
                                     stop=False)
                for jt in range(NT):
                    nc.tensor.matmul(out=ps_pot[orow, :],
                                     lhsT=t_q16[sys][jt][:],
                                     rhs=t_M[sys * NT + jt][:],
                                     start=False, stop=False)
                for ch in range(NCH):
                    gc = sys * NCH + ch
                    nc.tensor.matmul(out=ps_pot[orow, :], lhsT=t_Y[gc][:],
                                     rhs=t_oneh[:, gc * N:(gc + 1) * N],
                                     start=False, stop=(ch == NCH - 1))
                # combine: out = pot * q, then output DMA per system
                t_out = wp.tile([D, N], F32, tag=f"outf{sys}",
                                name=f"outf{sys}")
                nc.vector.tensor_tensor(out=t_out[:], in0=ps_pot[orow, :],
                                        in1=t_qT[orow, :], op=AOP.mult)
                nc.sync.dma_start(out=out[orow, :], in_=t_out[:])

    nc.compile()
    return nc


def _host_inputs(features, positions, cells, neighbor_indices,
                 neighbor_distances, W, b):
    features = np.asarray(features, np.float32)
    positions = np.asarray(positions, np.float32)
    cells = np.asarray(cells, np.float32)
    nidx = np.asarray(neighbor_indices)
    ndist = np.asarray(neighbor_distances, np.float32).reshape(S, E)
    W = np.asarray(W, np.float32)
    b = np.asarray(b, np.float32)

    assert np.allclose(cells, LCELL * np.eye(3, dtype=np.float32)[None]), \
        "kernel specialized to cubic L=8 cells"

    nh = _half_kgrid()
    K0 = len(nh)
    assert K0 <= K2 - 1
    ksq = (2.0 * PI / LCELL) ** 2 * (nh * nh).sum(1).astype(np.float64)
    vol = LCELL ** 3
    bgov = PREF * float(PI * SMEAR**2 / vol)
    G = 2.0 * PREF * (4.0 * PI / ksq) * np.exp(-0.5 * SMEAR**2 * ksq) / vol
    Gpad = np.zeros(K2, np.float64)
    Gpad[:K0] = G
    Gpad[K0] = -bgov  # background term via the k=0 pad slot (c=1, s=0)
    G2row = np.concatenate([Gpad, Gpad]).astype(np.float16)
    G2 = np.broadcast_to(G2row[None, :], (128, 2 * K2))

    per_sys, R0, NCH = _sr_arrange(nidx, ndist)
    SC = SYS_PER_CORE
    NCH2 = SC * NCH
    WSR = 8 * R0 + NCH2
    MW = 2 * K2 + D + NCH2 * 128

    nt3 = np.zeros((3, K2), np.float16)
    nt3[:, :K0] = nh.T.astype(np.float16)
    nt6 = np.concatenate([nt3, nt3], 0)    # [6, K2]
    WT_aug = np.concatenate([W.T, b[None, :]], 0).astype(np.float16)  # [65, 64]
    nid = np.concatenate([-np.eye(128), np.eye(128)], 1).astype(np.float16)

    in_maps = []
    for core in range(NCORES):
        s0 = core * SC
        fa = []
        p6 = []
        for s in range(s0, s0 + SC):
            f = features[s * N:(s + 1) * N].T                      # [64, 512]
            fa.append(np.concatenate([f, np.ones((1, N), np.float32)], 0))
            pf = (positions[s].T.astype(np.float64)) / LCELL       # [3, 512]
            ph = pf.astype(np.float16)
            pl = (pf - ph.astype(np.float64)).astype(np.float16)
            p6.append(np.concatenate([ph, pl], 0))                 # [6, 512]
        p6n6 = np.concatenate(p6 + [nt6], 1).astype(np.float16)    # [6, 1024+K2]

        srd_c = np.full((128, WSR), PAD_DIST, np.float16)
        sri_c = np.full((128, 8 * R0), -1, np.int16)
        misc = np.zeros((128, MW), np.float16)
        misc[:, 0:2 * K2] = G2
        misc[0:D + 1, 2 * K2:2 * K2 + D] = WT_aug
        oneh = np.zeros((128, NCH2 * N), np.float16)
        for sys_local in range(SC):
            s = s0 + sys_local
            (js, is_, ds_), (js2, is2, ds2) = per_sys[s]
            cnt = np.bincount(js, minlength=N)
            start = np.concatenate([[0], np.cumsum(cnt)[:-1]])
            slot = np.arange(len(js)) - start[js]
            blk = sys_local * 4 + js // 128
            row = js % 128
            col = blk * R0 + slot
            srd_c[row, col] = ds_.astype(np.float16)
            sri_c[row, col] = is_.astype(np.int16)
            for e in range(len(js2)):
                ch = e // 128
                sl = e % 128
                gc = sys_local * NCH + ch
                srd_c[sl, 8 * R0 + gc] = np.float16(ds2[e])
                misc[0:D, 2 * K2 + D + gc * 128 + sl] = \
                    features[s * N + js2[e]].astype(np.float16)
                misc[D, 2 * K2 + D + gc * 128 + sl] = 1.0
                oneh[sl, gc * N + is2[e]] = 1.0

        m = {
            "srd": srd_c,
            "p6n6": p6n6,
            "sri": sri_c,
            "nid": nid,
            "misc": misc,
            "featT": np.concatenate(fa, 1).astype(np.float16),
            "oneh": oneh,
        }
        in_maps.append(m)
    return in_maps, R0, NCH


def kernel(features, positions, cells, neighbor_indices, neighbor_distances,
           W, b, _trace=False):
    in_maps, R0, NCH = _host_inputs(features, positions, cells,
                                    neighbor_indices, neighbor_distances, W, b)
    key = (R0, NCH)
    if key not in _CACHE:
        _CACHE[key] = _build_nc(R0, NCH)
    nc = _CACHE[key]
    res = bass_utils.run_bass_kernel_spmd(nc, in_maps,
                                          core_ids=list(range(NCORES)),
                                          trace=_trace)
    blocks = []
    for i in range(NCORES):
        o = res.results[i]["out"]  # [SC*D, N] transposed per system
        for sys in range(SYS_PER_CORE):
            blocks.append(o[sys * D:(sys + 1) * D, :].T)
    out = np.concatenate(blocks, 0)
    if _trace:
        kernel.last_result = res
    return np.ascontiguousarray(out, dtype=np.float32)


def measure_hw_ns(features, positions, cells, neighbor_indices,
                  neighbor_distances, W, b, reps=300):
    """Time the kernel on hardware via an on-device repeat loop (amortizes
    the multi-ms axon RPC dispatch overhead). Returns per-iteration ns."""
    import time
    import jax
    from jax.sharding import Mesh, PartitionSpec, NamedSharding
    from jax.experimental.shard_map import shard_map
    from concourse import bass2jax
    from concourse.bass2jax import _bass_exec_p, partition_id_tensor

    bass2jax.install_neuronx_cc_hook()
    in_maps, R0, NCH = _host_inputs(features, positions, cells,
                                    neighbor_indices, neighbor_distances, W, b)

    def build_fn(nc, mesh, sh):
        partition_name = (nc.partition_id_tensor.name
                          if nc.partition_id_tensor else None)
        in_names, out_names, out_avals, zero_outs = [], [], [], []
        for alloc in nc.m.functions[0].allocations:
            if not isinstance(alloc, mybir.MemoryLocationSet):
                continue
            name = alloc.memorylocations[0].name
            if alloc.kind == "ExternalInput":
                if name != partition_name:
                    in_names.append(name)
            elif alloc.kind == "ExternalOutput":
                shape = tuple(alloc.tensor_shape)
                dtype = mybir.dt.np(alloc.dtype)
                out_names.append(name)
                out_avals.append(jax.core.ShapedArray(shape, dtype))
                zero_outs.append(np.zeros(shape, dtype))
        n_params = len(in_names)
        all_names = in_names + out_names
        if partition_name is not None:
            all_names = all_names + [partition_name]

        def _body(*args):
            operands = list(args)
            if partition_name is not None:
                operands.append(partition_id_tensor())
            return tuple(_bass_exec_p.bind(
                *operands, out_avals=tuple(out_avals), in_names=tuple(all_names),
                out_names=tuple(out_names), lowering_input_output_aliases=(),
                sim_require_finite=True, sim_require_nnan=True, nc=nc))

        specs_in = (PartitionSpec("core"),) * (n_params + len(out_names))
        specs_out = (PartitionSpec("core"),) * len(out_names)
        fn = jax.jit(shard_map(_body, mesh=mesh, in_specs=specs_in,
                               out_specs=specs_out, check_rep=False),
                     keep_unused=True)
        cat = [np.concatenate([np.asarray(in_maps[c][in_names[i]])
                               for c in range(NCORES)], 0)
               for i in range(n_params)]
        cat += [np.zeros((NCORES * z.shape[0], *z.shape[1:]), z.dtype)
                for z in zero_outs]
        dev = [jax.device_put(a, sh) for a in cat]
        return fn, dev

    devices = jax.devices()[:NCORES]
    mesh = Mesh(np.asarray(devices), ("core",))
    sh = NamedSharding(mesh, PartitionSpec("core"))

    def time_min(fn, dev, n=8):
        o = fn(*dev); jax.block_until_ready(o)
        best = float("inf")
        for _ in range(n):
            t0 = time.perf_counter()
            o = fn(*dev); jax.block_until_ready(o)
            best = min(best, (time.perf_counter() - t0) * 1e9)
        return best

    key1 = (R0, NCH)
    if key1 not in _CACHE:
        _CACHE[key1] = _build_nc(R0, NCH)
    fn1, dev1 = build_fn(_CACHE[key1], mesh, sh)
    t1 = time_min(fn1, dev1)
    keyr = (R0, NCH, reps)
    if keyr not in _CACHE:
        _CACHE[keyr] = _build_nc(R0, NCH, reps=reps)
    fnr, devr = build_fn(_CACHE[keyr], mesh, sh)
    tr = time_min(fnr, devr)
    return (tr - t1) / (reps - 1)
